# revision 35
# baseline (speedup 1.0000x reference)
"""AnomalyAttention Trainium2 kernel — 8-core SPMD, no collectives.

Problem: B=4, T=1024, D=512, H=8, DH=64.
  q/k/v = x@W (+b); logits = q@k^T/8; series = softmax(logits)
  sigma = softplus(x@Ws+bs)+1e-6; prior = rownorm(exp(-dist2/(2*(sigma^2+1e-6))))
  out = (series@v reshaped) @ Wo + bo
Returns (out, series, prior, sigma).

Sharding: core c handles batch b=c//2 and query-row half h2=c%2 (512 rows).
Each core computes k/v for the full T of its batch (recompute instead of
collective), so the 8 cores are fully independent.

Per-core dataflow (all engines via the Tile framework):
  - projections: bf16 matmuls (x^T and weights pre-cast on host);
    q^T/k^T stored f32r (the psum-copy rounds), v stored bf16 [s,d];
    projection chunks are emitted interleaved with the head pipeline
  - logits: f32r matmul, N=512 (full TensorE rate; 1/sqrt(dh) folded into Wq)
  - exp: ACT with fused row-sum (accum_out); series exp -> bf16
  - normalize+transpose of S' fused into one TensorE matmul per 128x128
    chunk: St = S'^T @ diag(1/rowsum); series normalized on DVE
  - prior: computed only on a 384-wide diagonal band (exact: off-band
    underflows to 0 in f32, sigma<8.8); ACT exp(nd2 * scale_ap) with fused
    row-sum, DVE normalize; band scattered into zeros on the host
  - S@v: head pairs packed into one psum via col tile_position; out = A^T@Wo
  - head pairs software-pipelined (pair p's logits/exp emitted before pair
    p-1's transpose+S@v) so ACT/DVE/TensorE overlap; softplus = ln(exp(z)+1)
    keeps every ACT op in the natural_log_exp_and_others table set
"""

import os
import sys

sys.path.insert(0, "/opt/trn_rl_repo")

import numpy as np
import ml_dtypes

import concourse.bass as bass
import concourse.mybir as mybir
import concourse.tile as tile
from concourse import bacc
from concourse.bass_utils import run_bass_kernel_spmd
from concourse.masks import make_identity

F32 = mybir.dt.float32
F32R = mybir.dt.float32r
BF16 = mybir.dt.bfloat16

B, T, D, H = 4, 1024, 512, 8
DH = D // H          # 64
TH = T // 2          # 512 rows per core
KC = D // 128        # 4 contraction chunks
NTC = TH // 128      # 4 query-row chunks per core
NSC = T // 128       # 8 key-row chunks
BW = 384             # prior band width (|t-s| >= 128 underflows to exactly 0)
AF = mybir.ActivationFunctionType

_NC_CACHE = {}
LAST_RESULTS = None  # test harness reads exec_time_ns from here


def _build():
    nc = bacc.Bacc("TRN2", target_bir_lowering=False, debug=False, num_devices=8)

    xt_bf16 = nc.declare_dram_parameter("xt_bf16", [D, T], BF16, isOutput=False)
    xtq_f32 = nc.declare_dram_parameter("xtq_f32", [D, TH], F32, isOutput=False)
    xtq_bf16 = nc.declare_dram_parameter("xtq_bf16", [D, TH], BF16, isOutput=False)
    wq = nc.declare_dram_parameter("wq", [D, D], BF16, isOutput=False)
    wk = nc.declare_dram_parameter("wk", [D, D], BF16, isOutput=False)
    wv = nc.declare_dram_parameter("wv", [D, D], BF16, isOutput=False)
    wo = nc.declare_dram_parameter("wo", [D, D], BF16, isOutput=False)
    ws = nc.declare_dram_parameter("ws", [D, H], F32, isOutput=False)
    bq = nc.declare_dram_parameter("bq", [128, KC], F32, isOutput=False)
    bk = nc.declare_dram_parameter("bk", [128, KC], F32, isOutput=False)
    bv = nc.declare_dram_parameter("bv", [1, D], BF16, isOutput=False)
    bo = nc.declare_dram_parameter("bo", [1, D], BF16, isOutput=False)
    bs_col = nc.declare_dram_parameter("bs_col", [H, 1], F32, isOutput=False)
    bs_row = nc.declare_dram_parameter("bs_row", [1, H], F32, isOutput=False)
    nd2 = nc.declare_dram_parameter("nd2", [TH, BW], F32, isOutput=False)

    o_series = nc.declare_dram_parameter("series", [H, TH, T], F32, isOutput=True)
    o_prior = nc.declare_dram_parameter("prior", [H, TH, BW], F32, isOutput=True)
    o_out = nc.declare_dram_parameter("out", [TH, D], F32, isOutput=True)
    o_sigma = nc.declare_dram_parameter("sigma", [H, TH], F32, isOutput=True)

    with tile.TileContext(nc) as tc:
        with (
            tc.tile_pool(name="per", bufs=1) as per,        # persistent tensors
            tc.tile_pool(name="work", bufs=2) as work,      # per-head rotating
            tc.tile_pool(name="stage", bufs=4) as stage,    # DMA-out staging
            tc.tile_pool(name="pbig", bufs=2, space="PSUM") as pbig,   # [128,1024]
            tc.tile_pool(name="pmid", bufs=3, space="PSUM") as pmid,   # [128,512]
        ):
            # ---- persistent loads: one DMA per tensor -----------------
            # [D, X] dram tensors load as [128, KC*X] tiles ("(c p) x -> p (c x)");
            # chunk k is the view [:, k*X:(k+1)*X]
            def load_chunked(handle, X, dt, nm, nchunk=KC, eng=None):
                tl = per.tile([128, nchunk * X], dt, name=nm, tag=nm)
                (eng or nc.sync).dma_start(
                    tl[:].rearrange("p (c x) -> p c x", c=nchunk),
                    handle.ap().rearrange("(c p) x -> p c x", p=128),
                )
                return [tl[:, k * X:(k + 1) * X] for k in range(nchunk)]

            def load_chunked_split(handle, X, dt, nm):
                tl = per.tile([128, KC * X], dt, name=nm, tag=nm)
                views = [tl[:, k * X:(k + 1) * X] for k in range(KC)]
                for k in range(KC):
                    nc.sync.dma_start(views[k], handle[bass.ts(k, 128), :])
                return views

            # q-side loads on sync queues, k/v-side on gpsimd queues so the
            # first projections start while the rest of the inputs stream in
            def load_pair_interleaved(h1, X1, nm1, h2, X2, nm2):
                t1 = per.tile([128, KC * X1], BF16, name=nm1, tag=nm1)
                t2 = per.tile([128, KC * X2], BF16, name=nm2, tag=nm2)
                v1 = [t1[:, k * X1:(k + 1) * X1] for k in range(KC)]
                v2 = [t2[:, k * X2:(k + 1) * X2] for k in range(KC)]
                for k in range(KC):
                    nc.sync.dma_start(v1[k], h1[bass.ts(k, 128), :])
                    nc.sync.dma_start(v2[k], h2[bass.ts(k, 128), :])
                return v1, v2

            wqt, xqb = load_pair_interleaved(wq, D, "wqt", xtq_bf16, TH, "xqbt")
            wkt = load_chunked(wk, D, BF16, "wkt", eng=nc.gpsimd)
            xtb = load_chunked(xt_bf16, T, BF16, "xtbt", eng=nc.gpsimd)
            wst = load_chunked(ws, H, F32, "wst", eng=nc.gpsimd)
            xqf = load_chunked(xtq_f32, TH, F32, "xqft", eng=nc.gpsimd)
            wvt = load_chunked(wv, D, BF16, "wvt", eng=nc.gpsimd)
            nd2t = load_chunked(nd2, BW, F32, "nd2t", nchunk=NTC, eng=nc.gpsimd)
            wot = load_chunked(wo, D, BF16, "wot", eng=nc.gpsimd)
            bqt = per.tile([128, KC], F32, name="bq", tag="bq")
            bkt = per.tile([128, KC], F32, name="bk", tag="bk")
            bvt = per.tile([1, D], BF16, name="bv", tag="bv")
            bot = per.tile([1, D], BF16, name="bo", tag="bo")
            bsc = per.tile([H, 1], F32, name="bsc", tag="bsc")
            bsr = per.tile([1, H], F32, name="bsr", tag="bsr")
            nc.sync.dma_start(bqt[:], bq[:])
            nc.sync.dma_start(bkt[:], bk[:])
            nc.sync.dma_start(bvt[:], bv[:])
            nc.sync.dma_start(bot[:], bo[:])
            nc.sync.dma_start(bsc[:], bs_col[:])
            nc.sync.dma_start(bsr[:], bs_row[:])

            eye = per.tile([128, 128], BF16, name="eye", tag="eye")
            make_identity(nc, eye[:])
            ones_b = per.tile([1, 128], BF16, name="ones_b", tag="ones_b")
            nc.vector.memset(ones_b[:], 1.0)
            ones_f = per.tile([1, 128], F32, name="ones_f", tag="ones_f")
            nc.vector.memset(ones_f[:], 1.0)
            # pin the natural_log_exp_and_others ACT table set (has both exp
            # and ln) before any Exp, so walrus never switches sets mid-kernel
            tpin = per.tile([1, 1], F32, name="tpin", tag="tpin")
            nc.scalar.activation(tpin[:], ones_f[0:1, 0:1], AF.Ln)

            # ---- projections (emitted interleaved with head pairs) ----
            # q^T [dout, t_half] (f32r), k^T [dout, s_full] (f32r)
            qT = [per.tile([128, TH], F32R, name=f"qT{m}", tag=f"qT{m}") for m in range(KC)]
            kT = [per.tile([128, T], F32R, name=f"kT{m}", tag=f"kT{m}") for m in range(KC)]
            vt = [per.tile([128, D], BF16, name=f"v{s}", tag=f"v{s}") for s in range(NSC)]

            def proj_qk(m):
                ps = pmid.tile([128, 512], F32, name="mid", tag="mid")
                for k in range(KC):
                    nc.tensor.matmul(
                        ps[:], wqt[k][:, bass.ts(m, 128)], xqb[k][:],
                        start=(k == 0), stop=(k == KC - 1),
                    )
                nc.vector.tensor_scalar_add(qT[m][:], ps[:], bqt[:, m:m + 1])
                for sh in range(2):
                    ps2 = pmid.tile([128, 512], F32, name="mid", tag="mid")
                    for k in range(KC):
                        nc.tensor.matmul(
                            ps2[:], wkt[k][:, bass.ts(m, 128)],
                            xtb[k][:, bass.ts(sh, 512)],
                            start=(k == 0), stop=(k == KC - 1),
                        )
                    nc.vector.tensor_scalar_add(
                        kT[m][:, bass.ts(sh, 512)], ps2[:], bkt[:, m:m + 1]
                    )

            def proj_v(s):
                ps = pmid.tile([128, 512], F32, name="mid", tag="mid")
                for k in range(KC):
                    nc.tensor.matmul(
                        ps[:], xtb[k][:, bass.ts(s, 128)], wvt[k][:],
                        start=(k == 0), stop=False,
                    )
                nc.tensor.matmul(ps[:], ones_b[:], bvt[:], start=False, stop=True)
                nc.vector.tensor_copy(vt[s][:], ps[:])

            # ---- sigma (both orientations) ----------------------------
            # row orientation [H, TH] for the sigma output
            ps = pmid.tile([H, 512], F32, name="mid", tag="mid")
            for k in range(KC):
                nc.tensor.matmul(
                    ps[:], wst[k][:], xqf[k][:],
                    start=(k == 0), stop=(k == KC - 1),
                )
            # softplus(z) = ln(exp(z)+1); all Exps grouped before all Lns so
            # the ACT table set (natural_log_exp_and_others) loads once
            ez_row = per.tile([H, TH], F32, name="ez_row", tag="ez_row")
            nc.scalar.activation(ez_row[:], ps[:], AF.Exp, bias=bsc[:, 0:1])
            # natural orientation [t, H] -> inv2s2 = 1/(2*((sp+1e-6)^2+1e-6))
            inv2s2 = [per.tile([128, H], F32, name=f"i2s{t}", tag=f"i2s{t}") for t in range(NTC)]
            ezn = [per.tile([128, H], F32, name=f"ezn{t}", tag=f"ezn{t}") for t in range(NTC)]
            for t in range(NTC):
                psn = pmid.tile([128, 512], F32, name="mid", tag="mid")
                for k in range(KC):
                    nc.tensor.matmul(
                        psn[:, 0:H], xqf[k][:, bass.ts(t, 128)], wst[k][:],
                        start=(k == 0), stop=False,
                    )
                nc.tensor.matmul(psn[:, 0:H], ones_f[:], bsr[:], start=False, stop=True)
                nc.scalar.activation(ezn[t][:], psn[:, 0:H], AF.Exp)
            sg_row = per.tile([H, TH], F32, name="sg_row", tag="sg_row")
            nc.scalar.activation(sg_row[:], ez_row[:], AF.Ln, bias=1.0)
            sig_row = per.tile([H, TH], F32, name="sig_row", tag="sig_row")
            nc.vector.tensor_scalar_add(sig_row[:], sg_row[:], 1e-6)
            nc.sync.dma_start(o_sigma[:], sig_row[:])
            for t in range(NTC):
                sp = per.tile([128, H], F32, name=f"sp{t}", tag=f"sp{t}")
                nc.scalar.activation(sp[:], ezn[t][:], AF.Ln, bias=1.0)
                nc.vector.tensor_scalar_add(sp[:], sp[:], 1e-6)
                sq = per.tile([128, H], F32, name=f"sq{t}", tag=f"sq{t}")
                nc.vector.tensor_tensor(sq[:], sp[:], sp[:], mybir.AluOpType.mult)
                nc.vector.tensor_scalar(
                    sq[:], sq[:], 1e-6, 2.0,
                    mybir.AluOpType.add, mybir.AluOpType.mult,
                )
                nc.vector.reciprocal(inv2s2[t][:], sq[:])

            # ---- per-head-pair attention + prior ----------------------
            # heads (2m, 2m+1) live in qT[m]/kT[m] at partition rows 0:64/64:128
            at = [per.tile([128, TH], BF16, name=f"at{m}", tag=f"at{m}") for m in range(KC)]

            def phase_logits(hp):
                """logits -> exp(+rowsum) -> series/prior normalize + DMA.
                Returns the pair's S' and diag tiles for phase_sv."""
                sp_t = [[None] * NTC for _ in range(2)]
                dg = [[None] * NTC for _ in range(2)]
                sst_p = [None, None]
                pst_p = [None, None]
                for t in range(NTC):
                    for side in range(2):
                        h, hr = 2 * hp + side, side * 64
                        lp = pbig.tile([128, T], F32, name="big", tag="big")
                        for sh in range(2):
                            nc.tensor.matmul(
                                lp[:, bass.ts(sh, 512)],
                                qT[hp][hr:hr + 64, bass.ts(t, 128)],
                                kT[hp][hr:hr + 64, bass.ts(sh, 512)],
                                start=True, stop=True,
                            )
                        spt = work.tile([128, T], BF16, name=f"sprime{side}_{t}", tag=f"sprime{side}_{t}")
                        sp_t[side][t] = spt
                        rs = work.tile([128, 1], F32, name=f"rs{side}_{t}", tag=f"rs{side}_{t}")
                        nc.scalar.activation(spt[:], lp[:], AF.Exp, accum_out=rs[:])
                        rc = work.tile([128, 1], F32, name=f"rc{side}_{t}", tag=f"rc{side}_{t}")
                        nc.vector.reciprocal(rc[:], rs[:])
                        dgt = work.tile([128, 128], BF16, name=f"diag{side}_{t}", tag=f"diag{side}_{t}")
                        dg[side][t] = dgt
                        nc.vector.tensor_scalar_mul(dgt[:], eye[:], rc[:, 0:1])
                        if t % 2 == 0:
                            sst_p[side] = stage.tile([128, 2 * T], F32, name="series_st", tag="series_st", bufs=2)
                        sst = sst_p[side]
                        nc.vector.tensor_scalar_mul(
                            sst[:, (t % 2) * T:(t % 2 + 1) * T], spt[:], rc[:, 0:1])
                        if t % 2 == 1:
                            nc.sync.dma_start(
                                o_series[h, (t - 1) * 128:(t + 1) * 128, :]
                                .rearrange("(c p) x -> p c x", p=128),
                                sst[:].rearrange("p (c x) -> p c x", c=2),
                            )
                        # prior for this (h, t-chunk), band only
                        pp = work.tile([128, BW], BF16, name=f"pp{side}_{t}", tag=f"pp{side}_{t}")
                        prs = work.tile([128, 1], F32, name=f"prs{side}_{t}", tag=f"prs{side}_{t}")
                        nc.scalar.activation(
                            pp[:], nd2t[t][:], AF.Exp,
                            scale=inv2s2[t][:, h:h + 1], accum_out=prs[:],
                        )
                        nc.vector.tensor_scalar_add(prs[:], prs[:], 1e-9)
                        prc = work.tile([128, 1], F32, name=f"prc{side}_{t}", tag=f"prc{side}_{t}")
                        nc.vector.reciprocal(prc[:], prs[:])
                        if t % 2 == 0:
                            pst_p[side] = stage.tile([128, 2 * BW], F32, name="prior_st", tag="prior_st", bufs=3)
                        pst = pst_p[side]
                        nc.vector.tensor_scalar_mul(
                            pst[:, (t % 2) * BW:(t % 2 + 1) * BW], pp[:], prc[:, 0:1])
                        if t % 2 == 1:
                            nc.sync.dma_start(
                                o_prior[h, (t - 1) * 128:(t + 1) * 128, :]
                                .rearrange("(c p) x -> p c x", p=128),
                                pst[:].rearrange("p (c x) -> p c x", c=2),
                            )
                return sp_t, dg

            def phase_sv(hp, sp_t, dg):
                """normalized transpose St = S'^T @ diag(1/rowsum), then the
                pair's S@v packed into one psum via col tile_position"""
                st = [[None] * NSC for _ in range(2)]
                ohp = pmid.tile([128, 512], F32, name="ohp", tag="oh", bufs=1)

                def sv(s):
                    for side in range(2):
                        h = 2 * hp + side
                        nc.tensor.matmul(
                            ohp[side * 64:side * 64 + 64, :],
                            vt[s][:, h * 64:h * 64 + 64], st[side][s][:],
                            start=(s == 0), stop=(s == NSC - 1),
                            skip_group_check=True,
                            tile_position=(0, side * 64),
                        )

                for s in range(NSC):
                    for side in range(2):
                        tp = pmid.tile([128, 512], F32, name="tp", tag="mid")
                        for t in range(NTC):
                            nc.tensor.matmul(
                                tp[:, bass.ts(t, 128)],
                                sp_t[side][t][:, bass.ts(s, 128)], dg[side][t][:],
                                start=True, stop=True, skip_group_check=True,
                            )
                        stt = work.tile([128, TH], BF16, name=f"st{side}_{s}", tag=f"st{side}_{s}")
                        st[side][s] = stt
                        if s % 4 != 3:
                            nc.vector.tensor_copy(stt[:], tp[:])
                        else:
                            nc.scalar.copy(stt[:], tp[:])
                    # S@v runs one s-chunk behind the transposes so the PE
                    # never stalls on the St psum->sbuf copy drain
                    if s > 0:
                        sv(s - 1)
                sv(NSC - 1)
                nc.vector.tensor_copy(at[hp][:], ohp[:])

            # software pipeline: only chunk 0 of q/k is projected up
            # front; remaining projections and v interleave with the head
            # pairs, and pair hp's logits/exp are emitted before pair
            # hp-1's transpose+S@v so every engine always has queued work
            proj_qk(0)
            prev = None
            for hp in range(H // 2):
                cur = (hp, *phase_logits(hp))
                if hp + 1 < KC:
                    proj_qk(hp + 1)
                if hp == 0:
                    for sc in range(NSC):
                        proj_v(sc)
                if prev is not None:
                    phase_sv(*prev)
                prev = cur
            phase_sv(*prev)

            # ---- output projection ------------------------------------
            ofull = stage.tile([128, NTC * D], F32, name="out_st", tag="out_st", bufs=1)
            for t in range(NTC):
                ps = pmid.tile([128, 512], F32, name="mid", tag="mid")
                for k in range(KC):
                    nc.tensor.matmul(
                        ps[:], at[k][:, bass.ts(t, 128)], wot[k][:],
                        start=(k == 0), stop=False,
                    )
                nc.tensor.matmul(ps[:], ones_b[:], bot[:], start=False, stop=True)
                nc.vector.tensor_copy(ofull[:, bass.ts(t, D)], ps[:])
            nc.sync.dma_start(
                o_out.ap().rearrange("(c p) x -> p c x", p=128),
                ofull[:].rearrange("p (c x) -> p c x", c=NTC),
            )

    nc.compile()
    return nc


def kernel(x, Wq_w, Wq_b, Wk_w, Wk_b, Wv_w, Wv_b, Ws_w, Ws_b, Wo_w, Wo_b):
    global LAST_RESULTS
    x = np.asarray(x, np.float32)
    Wq_w = np.asarray(Wq_w, np.float32); Wq_b = np.asarray(Wq_b, np.float32)
    Wk_w = np.asarray(Wk_w, np.float32); Wk_b = np.asarray(Wk_b, np.float32)
    Wv_w = np.asarray(Wv_w, np.float32); Wv_b = np.asarray(Wv_b, np.float32)
    Ws_w = np.asarray(Ws_w, np.float32); Ws_b = np.asarray(Ws_b, np.float32)
    Wo_w = np.asarray(Wo_w, np.float32); Wo_b = np.asarray(Wo_b, np.float32)

    if "nc" not in _NC_CACHE:
        _NC_CACHE["nc"] = _build()
    nc = _NC_CACHE["nc"]

    scale = 1.0 / np.sqrt(DH, dtype=np.float32)  # folded via q-side: 1/8
    wq_s = (Wq_w * scale).astype(ml_dtypes.bfloat16)
    bq_s = (Wq_b * scale).reshape(KC, 128).T.copy()
    wk_c = Wk_w.astype(ml_dtypes.bfloat16)
    bk_c = Wk_b.reshape(KC, 128).T.copy()
    wv_c = Wv_w.astype(ml_dtypes.bfloat16)
    bv_c = Wv_b.reshape(1, D).astype(ml_dtypes.bfloat16)
    wo_c = Wo_w.astype(ml_dtypes.bfloat16)
    bo_c = Wo_b.reshape(1, D).astype(ml_dtypes.bfloat16)
    bs_col = Ws_b.reshape(H, 1).astype(np.float32)
    bs_row = Ws_b.reshape(1, H).astype(np.float32)

    idx = np.arange(T, dtype=np.float32)

    def band_start(t0, tc):
        return min(max(t0 + tc * 128 - 128, 0), T - BW)

    def make_nd2_band(t0):
        nd2b = np.empty((TH, BW), np.float32)
        for tc in range(NTC):
            c0 = band_start(t0, tc)
            rows = t0 + tc * 128 + np.arange(128, dtype=np.float32)
            cols = c0 + np.arange(BW, dtype=np.float32)
            nd2b[tc * 128:(tc + 1) * 128, :] = -((rows[:, None] - cols[None, :]) ** 2)
        return nd2b

    in_maps = []
    for c in range(8):
        b, h2 = c // 2, c % 2
        t0 = h2 * TH
        xt = np.ascontiguousarray(x[b].T)
        in_maps.append({
            "xt_bf16": xt.astype(ml_dtypes.bfloat16),
            "xtq_f32": np.ascontiguousarray(xt[:, t0:t0 + TH]),
            "xtq_bf16": np.ascontiguousarray(xt[:, t0:t0 + TH]).astype(ml_dtypes.bfloat16),
            "wq": wq_s, "wk": wk_c, "wv": wv_c, "wo": wo_c,
            "ws": Ws_w,
            "bq": bq_s, "bk": bk_c, "bv": bv_c, "bo": bo_c,
            "bs_col": bs_col, "bs_row": bs_row,
            "nd2": make_nd2_band(t0),
        })

    trace = os.environ.get("BASS_KERNEL_TRACE") == "1"
    res = run_bass_kernel_spmd(nc, in_maps, core_ids=list(range(8)), trace=trace)
    LAST_RESULTS = res

    out = np.empty((B, T, D), np.float32)
    series = np.empty((B, H, T, T), np.float32)
    prior = np.zeros((B, H, T, T), np.float32)
    sigma = np.empty((B, H, T), np.float32)
    for c in range(8):
        b, h2 = c // 2, c % 2
        t0 = h2 * TH
        r = res.results[c]
        out[b, t0:t0 + TH, :] = r["out"]
        series[b, :, t0:t0 + TH, :] = r["series"]
        pb = r["prior"]
        for tc in range(NTC):
            c0 = band_start(t0, tc)
            g0 = t0 + tc * 128
            prior[b, :, g0:g0 + 128, c0:c0 + BW] = pb[:, tc * 128:(tc + 1) * 128, :]
        sigma[b, :, t0:t0 + TH] = r["sigma"]
    return out, series, prior, sigma


# revision 38
# speedup vs baseline: 1.0115x; 1.0115x over previous
"""AnomalyAttention Trainium2 kernel — 8-core SPMD, no collectives.

Problem: B=4, T=1024, D=512, H=8, DH=64.
  q/k/v = x@W (+b); logits = q@k^T/8; series = softmax(logits)
  sigma = softplus(x@Ws+bs)+1e-6; prior = rownorm(exp(-dist2/(2*(sigma^2+1e-6))))
  out = (series@v reshaped) @ Wo + bo
Returns (out, series, prior, sigma).

Sharding: core c handles batch b=c//2 and query-row half h2=c%2 (512 rows).
Each core computes k/v for the full T of its batch (recompute instead of
collective), so the 8 cores are fully independent.

Per-core dataflow (all engines via the Tile framework):
  - projections: bf16 matmuls (x^T and weights pre-cast on host);
    q^T/k^T stored f32r (the psum-copy rounds), v stored bf16 [s,d];
    projection chunks are emitted interleaved with the head pipeline
  - logits: f32r matmul, N=512 (full TensorE rate; 1/sqrt(dh) folded into Wq)
  - exp: ACT with fused row-sum (accum_out); series exp -> bf16
  - normalize+transpose of S' fused into one TensorE matmul per 128x128
    chunk: St = S'^T @ diag(1/rowsum); series normalized on DVE
  - prior: computed only on a 384-wide diagonal band (exact: off-band
    underflows to 0 in f32, sigma<8.8); ACT exp(nd2 * scale_ap) with fused
    row-sum, DVE normalize; band scattered into zeros on the host
  - S@v: head pairs packed into one psum via col tile_position; out = A^T@Wo
  - head pairs software-pipelined (pair p's logits/exp emitted before pair
    p-1's transpose+S@v) so ACT/DVE/TensorE overlap; softplus = ln(exp(z)+1)
    keeps every ACT op in the natural_log_exp_and_others table set
"""

import os
import sys

sys.path.insert(0, "/opt/trn_rl_repo")

import numpy as np
import ml_dtypes

import concourse.bass as bass
import concourse.mybir as mybir
import concourse.tile as tile
from concourse import bacc
from concourse.bass_utils import run_bass_kernel_spmd
from concourse.masks import make_identity

F32 = mybir.dt.float32
F32R = mybir.dt.float32r
BF16 = mybir.dt.bfloat16

B, T, D, H = 4, 1024, 512, 8
DH = D // H          # 64
TH = T // 2          # 512 rows per core
KC = D // 128        # 4 contraction chunks
NTC = TH // 128      # 4 query-row chunks per core
NSC = T // 128       # 8 key-row chunks
BW = 384             # prior band width (|t-s| >= 128 underflows to exactly 0)
AF = mybir.ActivationFunctionType

_NC_CACHE = {}
LAST_RESULTS = None  # test harness reads exec_time_ns from here


def _build():
    nc = bacc.Bacc("TRN2", target_bir_lowering=False, debug=False, num_devices=8)

    xt_bf16 = nc.declare_dram_parameter("xt_bf16", [D, T], BF16, isOutput=False)
    xtq_f32 = nc.declare_dram_parameter("xtq_f32", [D, TH], F32, isOutput=False)
    xtq_bf16 = nc.declare_dram_parameter("xtq_bf16", [D, TH], BF16, isOutput=False)
    wq = nc.declare_dram_parameter("wq", [D, D], BF16, isOutput=False)
    wk = nc.declare_dram_parameter("wk", [D, D], BF16, isOutput=False)
    wv = nc.declare_dram_parameter("wv", [D, D], BF16, isOutput=False)
    wo = nc.declare_dram_parameter("wo", [D, D], BF16, isOutput=False)
    ws = nc.declare_dram_parameter("ws", [D, H], F32, isOutput=False)
    bq = nc.declare_dram_parameter("bq", [128, KC], F32, isOutput=False)
    bk = nc.declare_dram_parameter("bk", [128, KC], F32, isOutput=False)
    bv = nc.declare_dram_parameter("bv", [1, D], BF16, isOutput=False)
    bo = nc.declare_dram_parameter("bo", [1, D], BF16, isOutput=False)
    bs_col = nc.declare_dram_parameter("bs_col", [H, 1], F32, isOutput=False)
    bs_row = nc.declare_dram_parameter("bs_row", [1, H], F32, isOutput=False)
    nd2 = nc.declare_dram_parameter("nd2", [TH, BW], F32, isOutput=False)

    o_series = nc.declare_dram_parameter("series", [H, TH, T], F32, isOutput=True)
    o_prior = nc.declare_dram_parameter("prior", [H, TH, BW], F32, isOutput=True)
    o_out = nc.declare_dram_parameter("out", [TH, D], F32, isOutput=True)
    o_sigma = nc.declare_dram_parameter("sigma", [H, TH], F32, isOutput=True)

    with tile.TileContext(nc) as tc:
        with (
            tc.tile_pool(name="per", bufs=1) as per,        # persistent tensors
            tc.tile_pool(name="work", bufs=2) as work,      # per-head rotating
            tc.tile_pool(name="stage", bufs=4) as stage,    # DMA-out staging
            tc.tile_pool(name="pbig", bufs=2, space="PSUM") as pbig,   # [128,1024]
            tc.tile_pool(name="pmid", bufs=3, space="PSUM") as pmid,   # [128,512]
        ):
            # ---- persistent loads: one DMA per tensor -----------------
            # [D, X] dram tensors load as [128, KC*X] tiles ("(c p) x -> p (c x)");
            # chunk k is the view [:, k*X:(k+1)*X]
            def load_chunked(handle, X, dt, nm, nchunk=KC, eng=None):
                tl = per.tile([128, nchunk * X], dt, name=nm, tag=nm)
                (eng or nc.sync).dma_start(
                    tl[:].rearrange("p (c x) -> p c x", c=nchunk),
                    handle.ap().rearrange("(c p) x -> p c x", p=128),
                )
                return [tl[:, k * X:(k + 1) * X] for k in range(nchunk)]

            def load_chunked_split(handle, X, dt, nm):
                tl = per.tile([128, KC * X], dt, name=nm, tag=nm)
                views = [tl[:, k * X:(k + 1) * X] for k in range(KC)]
                for k in range(KC):
                    nc.sync.dma_start(views[k], handle[bass.ts(k, 128), :])
                return views

            # q-side loads on sync queues, k/v-side on gpsimd queues so the
            # first projections start while the rest of the inputs stream in
            def load_pair_interleaved(h1, X1, nm1, h2, X2, nm2):
                t1 = per.tile([128, KC * X1], BF16, name=nm1, tag=nm1)
                t2 = per.tile([128, KC * X2], BF16, name=nm2, tag=nm2)
                v1 = [t1[:, k * X1:(k + 1) * X1] for k in range(KC)]
                v2 = [t2[:, k * X2:(k + 1) * X2] for k in range(KC)]
                for k in range(KC):
                    nc.sync.dma_start(v1[k], h1[bass.ts(k, 128), :])
                    nc.sync.dma_start(v2[k], h2[bass.ts(k, 128), :])
                return v1, v2

            wqt, xqb = load_pair_interleaved(wq, D, "wqt", xtq_bf16, TH, "xqbt")
            wkt = load_chunked(wk, D, BF16, "wkt", eng=nc.gpsimd)
            xtb = load_chunked(xt_bf16, T, BF16, "xtbt", eng=nc.gpsimd)
            early = tc.alloc_tile_pool(name="early", bufs=1)

            def load_chunked_early(handle, X, dt, nm, nchunk=KC):
                tl = early.tile([128, nchunk * X], dt, name=nm, tag=nm)
                nc.gpsimd.dma_start(
                    tl[:].rearrange("p (c x) -> p c x", c=nchunk),
                    handle.ap().rearrange("(c p) x -> p c x", p=128),
                )
                return [tl[:, k * X:(k + 1) * X] for k in range(nchunk)]

            wst = load_chunked_early(ws, H, F32, "wst")
            xqf = load_chunked_early(xtq_f32, TH, F32, "xqft")
            wvt = load_chunked(wv, D, BF16, "wvt", eng=nc.gpsimd)
            nd2t = load_chunked(nd2, BW, F32, "nd2t", nchunk=NTC, eng=nc.gpsimd)
            wot = load_chunked(wo, D, BF16, "wot", eng=nc.gpsimd)
            bqt = per.tile([128, KC], F32, name="bq", tag="bq")
            bkt = per.tile([128, KC], F32, name="bk", tag="bk")
            bvt = per.tile([1, D], BF16, name="bv", tag="bv")
            bot = per.tile([1, D], BF16, name="bo", tag="bo")
            bsc = per.tile([H, 1], F32, name="bsc", tag="bsc")
            bsr = per.tile([1, H], F32, name="bsr", tag="bsr")
            nc.sync.dma_start(bqt[:], bq[:])
            nc.sync.dma_start(bkt[:], bk[:])
            nc.sync.dma_start(bvt[:], bv[:])
            nc.sync.dma_start(bot[:], bo[:])
            nc.sync.dma_start(bsc[:], bs_col[:])
            nc.sync.dma_start(bsr[:], bs_row[:])

            eye = per.tile([128, 128], BF16, name="eye", tag="eye")
            make_identity(nc, eye[:])
            ones_b = per.tile([1, 128], BF16, name="ones_b", tag="ones_b")
            nc.vector.memset(ones_b[:], 1.0)
            ones_f = per.tile([1, 128], F32, name="ones_f", tag="ones_f")
            nc.vector.memset(ones_f[:], 1.0)
            # pin the natural_log_exp_and_others ACT table set (has both exp
            # and ln) before any Exp, so walrus never switches sets mid-kernel
            tpin = per.tile([1, 1], F32, name="tpin", tag="tpin")
            nc.scalar.activation(tpin[:], ones_f[0:1, 0:1], AF.Ln)

            # ---- projections (emitted interleaved with head pairs) ----
            # q^T [dout, t_half] (f32r), k^T [dout, s_full] (f32r)
            qT = [per.tile([128, TH], F32R, name=f"qT{m}", tag=f"qT{m}") for m in range(KC)]
            kT = [per.tile([128, T], F32R, name=f"kT{m}", tag=f"kT{m}") for m in range(KC)]
            vt = [per.tile([128, D], BF16, name=f"v{s}", tag=f"v{s}") for s in range(NSC)]

            def proj_qk(m):
                ps = pmid.tile([128, 512], F32, name="mid", tag="mid")
                for k in range(KC):
                    nc.tensor.matmul(
                        ps[:], wqt[k][:, bass.ts(m, 128)], xqb[k][:],
                        start=(k == 0), stop=(k == KC - 1),
                    )
                nc.vector.tensor_scalar_add(qT[m][:], ps[:], bqt[:, m:m + 1])
                for sh in range(2):
                    ps2 = pmid.tile([128, 512], F32, name="mid", tag="mid")
                    for k in range(KC):
                        nc.tensor.matmul(
                            ps2[:], wkt[k][:, bass.ts(m, 128)],
                            xtb[k][:, bass.ts(sh, 512)],
                            start=(k == 0), stop=(k == KC - 1),
                        )
                    nc.vector.tensor_scalar_add(
                        kT[m][:, bass.ts(sh, 512)], ps2[:], bkt[:, m:m + 1]
                    )

            def proj_v(s):
                ps = pmid.tile([128, 512], F32, name="mid", tag="mid")
                for k in range(KC):
                    nc.tensor.matmul(
                        ps[:], xtb[k][:, bass.ts(s, 128)], wvt[k][:],
                        start=(k == 0), stop=False,
                    )
                nc.tensor.matmul(ps[:], ones_b[:], bvt[:], start=False, stop=True)
                nc.vector.tensor_copy(vt[s][:], ps[:])

            # ---- sigma (both orientations) ----------------------------
            # row orientation [H, TH] for the sigma output
            ps = pmid.tile([H, 512], F32, name="mid", tag="mid")
            for k in range(KC):
                nc.tensor.matmul(
                    ps[:], wst[k][:], xqf[k][:],
                    start=(k == 0), stop=(k == KC - 1),
                )
            # softplus(z) = ln(exp(z)+1); all Exps grouped before all Lns so
            # the ACT table set (natural_log_exp_and_others) loads once
            ez_row = early.tile([H, TH], F32, name="ez_row", tag="ez_row")
            nc.scalar.activation(ez_row[:], ps[:], AF.Exp, bias=bsc[:, 0:1])
            # natural orientation [t, H] -> inv2s2 = 1/(2*((sp+1e-6)^2+1e-6))
            inv2s2 = [per.tile([128, H], F32, name=f"i2s{t}", tag=f"i2s{t}") for t in range(NTC)]
            ezn = [early.tile([128, H], F32, name=f"ezn{t}", tag=f"ezn{t}") for t in range(NTC)]
            for t in range(NTC):
                psn = pmid.tile([128, 512], F32, name="mid", tag="mid")
                for k in range(KC):
                    nc.tensor.matmul(
                        psn[:, 0:H], xqf[k][:, bass.ts(t, 128)], wst[k][:],
                        start=(k == 0), stop=False,
                    )
                nc.tensor.matmul(psn[:, 0:H], ones_f[:], bsr[:], start=False, stop=True)
                nc.scalar.activation(ezn[t][:], psn[:, 0:H], AF.Exp)
            nc.scalar.activation(ez_row[:], ez_row[:], AF.Ln, bias=1.0)
            nc.vector.tensor_scalar_add(ez_row[:], ez_row[:], 1e-6)
            nc.sync.dma_start(o_sigma[:], ez_row[:])
            for t in range(NTC):
                sp = ezn[t]
                nc.scalar.activation(sp[:], ezn[t][:], AF.Ln, bias=1.0)
                nc.vector.tensor_scalar_add(sp[:], sp[:], 1e-6)
                sq = early.tile([128, H], F32, name=f"sq{t}", tag=f"sq{t}")
                nc.vector.tensor_tensor(sq[:], sp[:], sp[:], mybir.AluOpType.mult)
                nc.vector.tensor_scalar(
                    sq[:], sq[:], 1e-6, 2.0,
                    mybir.AluOpType.add, mybir.AluOpType.mult,
                )
                nc.vector.reciprocal(inv2s2[t][:], sq[:])

            early.release()

            # ---- per-head-pair attention + prior ----------------------
            # heads (2m, 2m+1) live in qT[m]/kT[m] at partition rows 0:64/64:128
            at = [per.tile([128, TH], BF16, name=f"at{m}", tag=f"at{m}") for m in range(KC)]

            def phase_logits(hp):
                """logits -> exp(+rowsum) -> series/prior normalize + DMA.
                Returns the pair's S' and diag tiles for phase_sv."""
                sp_t = [[None] * NTC for _ in range(2)]
                dg = [[None] * NTC for _ in range(2)]
                sst_p = [None, None]
                pst_p = [None, None]
                for t in range(NTC):
                    for side in range(2):
                        h, hr = 2 * hp + side, side * 64
                        lp = pbig.tile([128, T], F32, name="big", tag="big")
                        for sh in range(2):
                            nc.tensor.matmul(
                                lp[:, bass.ts(sh, 512)],
                                qT[hp][hr:hr + 64, bass.ts(t, 128)],
                                kT[hp][hr:hr + 64, bass.ts(sh, 512)],
                                start=True, stop=True,
                            )
                        spt = work.tile([128, T], BF16, name=f"sprime{side}_{t}", tag=f"sprime{side}_{t}")
                        sp_t[side][t] = spt
                        rs = work.tile([128, 1], F32, name=f"rs{side}_{t}", tag=f"rs{side}_{t}")
                        nc.scalar.activation(spt[:], lp[:], AF.Exp, accum_out=rs[:])
                        rc = work.tile([128, 1], F32, name=f"rc{side}_{t}", tag=f"rc{side}_{t}")
                        nc.vector.reciprocal(rc[:], rs[:])
                        dgt = work.tile([128, 128], BF16, name=f"diag{side}_{t}", tag=f"diag{side}_{t}")
                        dg[side][t] = dgt
                        nc.vector.tensor_scalar_mul(dgt[:], eye[:], rc[:, 0:1])
                        if t % 2 == 0:
                            sst_p[side] = stage.tile([128, 2 * T], F32, name="series_st", tag="series_st", bufs=3)
                        sst = sst_p[side]
                        nc.vector.tensor_scalar_mul(
                            sst[:, (t % 2) * T:(t % 2 + 1) * T], spt[:], rc[:, 0:1])
                        if t % 2 == 1:
                            nc.sync.dma_start(
                                o_series[h, (t - 1) * 128:(t + 1) * 128, :]
                                .rearrange("(c p) x -> p c x", p=128),
                                sst[:].rearrange("p (c x) -> p c x", c=2),
                            )
                        # prior for this (h, t-chunk), band only
                        pp = work.tile([128, BW], BF16, name=f"pp{side}_{t}", tag=f"pp{side}_{t}")
                        prs = work.tile([128, 1], F32, name=f"prs{side}_{t}", tag=f"prs{side}_{t}")
                        nc.scalar.activation(
                            pp[:], nd2t[t][:], AF.Exp,
                            scale=inv2s2[t][:, h:h + 1], accum_out=prs[:],
                        )
                        nc.vector.tensor_scalar_add(prs[:], prs[:], 1e-9)
                        prc = work.tile([128, 1], F32, name=f"prc{side}_{t}", tag=f"prc{side}_{t}")
                        nc.vector.reciprocal(prc[:], prs[:])
                        if t % 2 == 0:
                            pst_p[side] = stage.tile([128, 2 * BW], F32, name="prior_st", tag="prior_st", bufs=3)
                        pst = pst_p[side]
                        nc.vector.tensor_scalar_mul(
                            pst[:, (t % 2) * BW:(t % 2 + 1) * BW], pp[:], prc[:, 0:1])
                        if t % 2 == 1:
                            nc.sync.dma_start(
                                o_prior[h, (t - 1) * 128:(t + 1) * 128, :]
                                .rearrange("(c p) x -> p c x", p=128),
                                pst[:].rearrange("p (c x) -> p c x", c=2),
                            )
                return sp_t, dg

            def phase_sv(hp, sp_t, dg):
                """normalized transpose St = S'^T @ diag(1/rowsum), then the
                pair's S@v packed into one psum via col tile_position"""
                st = [[None] * NSC for _ in range(2)]
                ohp = pmid.tile([128, 512], F32, name="ohp", tag="oh", bufs=1)

                def sv(s):
                    for side in range(2):
                        h = 2 * hp + side
                        nc.tensor.matmul(
                            ohp[side * 64:side * 64 + 64, :],
                            vt[s][:, h * 64:h * 64 + 64], st[side][s][:],
                            start=(s == 0), stop=(s == NSC - 1),
                            skip_group_check=True,
                            tile_position=(0, side * 64),
                        )

                for s in range(NSC):
                    for side in range(2):
                        tp = pmid.tile([128, 512], F32, name="tp", tag="mid")
                        for t in range(NTC):
                            nc.tensor.matmul(
                                tp[:, bass.ts(t, 128)],
                                sp_t[side][t][:, bass.ts(s, 128)], dg[side][t][:],
                                start=True, stop=True, skip_group_check=True,
                            )
                        stt = work.tile([128, TH], BF16, name=f"st{side}_{s}", tag=f"st{side}_{s}")
                        st[side][s] = stt
                        if s % 4 != 3:
                            nc.vector.tensor_copy(stt[:], tp[:])
                        else:
                            nc.scalar.copy(stt[:], tp[:])
                    # S@v runs one s-chunk behind the transposes so the PE
                    # never stalls on the St psum->sbuf copy drain
                    if s > 0:
                        sv(s - 1)
                sv(NSC - 1)
                nc.vector.tensor_copy(at[hp][:], ohp[:])

            # software pipeline: only chunk 0 of q/k is projected up
            # front; remaining projections and v interleave with the head
            # pairs, and pair hp's logits/exp are emitted before pair
            # hp-1's transpose+S@v so every engine always has queued work
            proj_qk(0)
            prev = None
            for hp in range(H // 2):
                cur = (hp, *phase_logits(hp))
                if hp + 1 < KC:
                    proj_qk(hp + 1)
                if hp == 0:
                    for sc in range(NSC):
                        proj_v(sc)
                if prev is not None:
                    phase_sv(*prev)
                prev = cur
            phase_sv(*prev)

            # ---- output projection ------------------------------------
            ofull = stage.tile([128, NTC * D], F32, name="out_st", tag="out_st", bufs=1)
            for t in range(NTC):
                ps = pmid.tile([128, 512], F32, name="mid", tag="mid")
                for k in range(KC):
                    nc.tensor.matmul(
                        ps[:], at[k][:, bass.ts(t, 128)], wot[k][:],
                        start=(k == 0), stop=False,
                    )
                nc.tensor.matmul(ps[:], ones_b[:], bot[:], start=False, stop=True)
                nc.vector.tensor_copy(ofull[:, bass.ts(t, D)], ps[:])
            nc.sync.dma_start(
                o_out.ap().rearrange("(c p) x -> p c x", p=128),
                ofull[:].rearrange("p (c x) -> p c x", c=NTC),
            )

    nc.compile()
    return nc


def kernel(x, Wq_w, Wq_b, Wk_w, Wk_b, Wv_w, Wv_b, Ws_w, Ws_b, Wo_w, Wo_b):
    global LAST_RESULTS
    x = np.asarray(x, np.float32)
    Wq_w = np.asarray(Wq_w, np.float32); Wq_b = np.asarray(Wq_b, np.float32)
    Wk_w = np.asarray(Wk_w, np.float32); Wk_b = np.asarray(Wk_b, np.float32)
    Wv_w = np.asarray(Wv_w, np.float32); Wv_b = np.asarray(Wv_b, np.float32)
    Ws_w = np.asarray(Ws_w, np.float32); Ws_b = np.asarray(Ws_b, np.float32)
    Wo_w = np.asarray(Wo_w, np.float32); Wo_b = np.asarray(Wo_b, np.float32)

    if "nc" not in _NC_CACHE:
        _NC_CACHE["nc"] = _build()
    nc = _NC_CACHE["nc"]

    scale = 1.0 / np.sqrt(DH, dtype=np.float32)  # folded via q-side: 1/8
    wq_s = (Wq_w * scale).astype(ml_dtypes.bfloat16)
    bq_s = (Wq_b * scale).reshape(KC, 128).T.copy()
    wk_c = Wk_w.astype(ml_dtypes.bfloat16)
    bk_c = Wk_b.reshape(KC, 128).T.copy()
    wv_c = Wv_w.astype(ml_dtypes.bfloat16)
    bv_c = Wv_b.reshape(1, D).astype(ml_dtypes.bfloat16)
    wo_c = Wo_w.astype(ml_dtypes.bfloat16)
    bo_c = Wo_b.reshape(1, D).astype(ml_dtypes.bfloat16)
    bs_col = Ws_b.reshape(H, 1).astype(np.float32)
    bs_row = Ws_b.reshape(1, H).astype(np.float32)

    idx = np.arange(T, dtype=np.float32)

    def band_start(t0, tc):
        return min(max(t0 + tc * 128 - 128, 0), T - BW)

    def make_nd2_band(t0):
        nd2b = np.empty((TH, BW), np.float32)
        for tc in range(NTC):
            c0 = band_start(t0, tc)
            rows = t0 + tc * 128 + np.arange(128, dtype=np.float32)
            cols = c0 + np.arange(BW, dtype=np.float32)
            nd2b[tc * 128:(tc + 1) * 128, :] = -((rows[:, None] - cols[None, :]) ** 2)
        return nd2b

    in_maps = []
    for c in range(8):
        b, h2 = c // 2, c % 2
        t0 = h2 * TH
        xt = np.ascontiguousarray(x[b].T)
        in_maps.append({
            "xt_bf16": xt.astype(ml_dtypes.bfloat16),
            "xtq_f32": np.ascontiguousarray(xt[:, t0:t0 + TH]),
            "xtq_bf16": np.ascontiguousarray(xt[:, t0:t0 + TH]).astype(ml_dtypes.bfloat16),
            "wq": wq_s, "wk": wk_c, "wv": wv_c, "wo": wo_c,
            "ws": Ws_w,
            "bq": bq_s, "bk": bk_c, "bv": bv_c, "bo": bo_c,
            "bs_col": bs_col, "bs_row": bs_row,
            "nd2": make_nd2_band(t0),
        })

    trace = os.environ.get("BASS_KERNEL_TRACE") == "1"
    res = run_bass_kernel_spmd(nc, in_maps, core_ids=list(range(8)), trace=trace)
    LAST_RESULTS = res

    out = np.empty((B, T, D), np.float32)
    series = np.empty((B, H, T, T), np.float32)
    prior = np.zeros((B, H, T, T), np.float32)
    sigma = np.empty((B, H, T), np.float32)
    for c in range(8):
        b, h2 = c // 2, c % 2
        t0 = h2 * TH
        r = res.results[c]
        out[b, t0:t0 + TH, :] = r["out"]
        series[b, :, t0:t0 + TH, :] = r["series"]
        pb = r["prior"]
        for tc in range(NTC):
            c0 = band_start(t0, tc)
            g0 = t0 + tc * 128
            prior[b, :, g0:g0 + 128, c0:c0 + BW] = pb[:, tc * 128:(tc + 1) * 128, :]
        sigma[b, :, t0:t0 + TH] = r["sigma"]
    return out, series, prior, sigma


# revision 39
# speedup vs baseline: 1.1947x; 1.1810x over previous
"""AnomalyAttention Trainium2 kernel — 8-core SPMD, no collectives.

Problem: B=4, T=1024, D=512, H=8, DH=64.
  q/k/v = x@W (+b); logits = q@k^T/8; series = softmax(logits)
  sigma = softplus(x@Ws+bs)+1e-6; prior = rownorm(exp(-dist2/(2*(sigma^2+1e-6))))
  out = (series@v reshaped) @ Wo + bo
Returns (out, series, prior, sigma).

Sharding: core c handles batch b=c//2 and query-row half h2=c%2 (512 rows).
Each core computes k/v for the full T of its batch (recompute instead of
collective), so the 8 cores are fully independent.

Per-core dataflow (all engines via the Tile framework):
  - projections: bf16 matmuls (x^T and weights pre-cast on host);
    q^T/k^T stored f32r (the psum-copy rounds), v stored bf16 [s,d];
    projection chunks are emitted interleaved with the head pipeline
  - logits: f32r matmul, N=512 (full TensorE rate; 1/sqrt(dh) folded into Wq)
  - exp: ACT with fused row-sum (accum_out); series exp -> bf16
  - normalize+transpose of S' fused into one TensorE matmul per 128x128
    chunk: St = S'^T @ diag(1/rowsum); series normalized on DVE
  - prior: computed only on a 384-wide diagonal band (exact: off-band
    underflows to 0 in f32, sigma<8.8); ACT exp(nd2 * scale_ap) with fused
    row-sum, DVE normalize; band scattered into zeros on the host
  - S@v: head pairs packed into one psum via col tile_position; out = A^T@Wo
  - head pairs software-pipelined (pair p's logits/exp emitted before pair
    p-1's transpose+S@v) so ACT/DVE/TensorE overlap; softplus = ln(exp(z)+1)
    keeps every ACT op in the natural_log_exp_and_others table set
"""

import os
import sys

sys.path.insert(0, "/opt/trn_rl_repo")

import numpy as np
import ml_dtypes

import concourse.bass as bass
import concourse.mybir as mybir
import concourse.tile as tile
from concourse import bacc
from concourse.bass_utils import run_bass_kernel_spmd
from concourse.masks import make_identity

F32 = mybir.dt.float32
F32R = mybir.dt.float32r
BF16 = mybir.dt.bfloat16

B, T, D, H = 4, 1024, 512, 8
DH = D // H          # 64
TH = T // 2          # 512 rows per core
KC = D // 128        # 4 contraction chunks
NTC = TH // 128      # 4 query-row chunks per core
NSC = T // 128       # 8 key-row chunks
BW = 384             # prior band width (|t-s| >= 128 underflows to exactly 0)
AF = mybir.ActivationFunctionType

_NC_CACHE = {}
LAST_RESULTS = None  # test harness reads exec_time_ns from here


def _build():
    nc = bacc.Bacc("TRN2", target_bir_lowering=False, debug=False, num_devices=8)

    xt_bf16 = nc.declare_dram_parameter("xt_bf16", [D, T], BF16, isOutput=False)
    xtq_f32 = nc.declare_dram_parameter("xtq_f32", [D, TH], F32, isOutput=False)
    xtq_bf16 = nc.declare_dram_parameter("xtq_bf16", [D, TH], BF16, isOutput=False)
    wq = nc.declare_dram_parameter("wq", [D, D], BF16, isOutput=False)
    wk = nc.declare_dram_parameter("wk", [D, D], BF16, isOutput=False)
    wv = nc.declare_dram_parameter("wv", [D, D], BF16, isOutput=False)
    wo = nc.declare_dram_parameter("wo", [D, D], BF16, isOutput=False)
    ws = nc.declare_dram_parameter("ws", [D, H], F32, isOutput=False)
    bq = nc.declare_dram_parameter("bq", [128, KC], F32, isOutput=False)
    bk = nc.declare_dram_parameter("bk", [128, KC], F32, isOutput=False)
    bv = nc.declare_dram_parameter("bv", [1, D], BF16, isOutput=False)
    bo = nc.declare_dram_parameter("bo", [1, D], BF16, isOutput=False)
    bs_col = nc.declare_dram_parameter("bs_col", [H, 1], F32, isOutput=False)
    bs_row = nc.declare_dram_parameter("bs_row", [1, H], F32, isOutput=False)
    nd2 = nc.declare_dram_parameter("nd2", [TH, BW], F32, isOutput=False)

    o_series = nc.declare_dram_parameter("series", [H, TH, T], F32, isOutput=True)
    o_prior = nc.declare_dram_parameter("prior", [H, TH, BW], F32, isOutput=True)
    o_out = nc.declare_dram_parameter("out", [TH, D], F32, isOutput=True)
    o_sigma = nc.declare_dram_parameter("sigma", [H, TH], F32, isOutput=True)

    with tile.TileContext(nc) as tc:
        with (
            tc.tile_pool(name="per", bufs=1) as per,        # persistent tensors
            tc.tile_pool(name="work", bufs=2) as work,      # per-head rotating
            tc.tile_pool(name="stage", bufs=4) as stage,    # DMA-out staging
            tc.tile_pool(name="pbig", bufs=2, space="PSUM") as pbig,   # [128,1024]
            tc.tile_pool(name="pmid", bufs=3, space="PSUM") as pmid,   # [128,512]
        ):
            # ---- persistent loads: one DMA per tensor -----------------
            # [D, X] dram tensors load as [128, KC*X] tiles ("(c p) x -> p (c x)");
            # chunk k is the view [:, k*X:(k+1)*X]
            def load_chunked(handle, X, dt, nm, nchunk=KC, eng=None):
                tl = per.tile([128, nchunk * X], dt, name=nm, tag=nm)
                (eng or nc.sync).dma_start(
                    tl[:].rearrange("p (c x) -> p c x", c=nchunk),
                    handle.ap().rearrange("(c p) x -> p c x", p=128),
                )
                return [tl[:, k * X:(k + 1) * X] for k in range(nchunk)]

            def load_chunked_split(handle, X, dt, nm):
                tl = per.tile([128, KC * X], dt, name=nm, tag=nm)
                views = [tl[:, k * X:(k + 1) * X] for k in range(KC)]
                for k in range(KC):
                    nc.sync.dma_start(views[k], handle[bass.ts(k, 128), :])
                return views

            # q-side loads on sync queues, k/v-side on gpsimd queues so the
            # first projections start while the rest of the inputs stream in
            def load_pair_interleaved(h1, X1, nm1, h2, X2, nm2):
                t1 = per.tile([128, KC * X1], BF16, name=nm1, tag=nm1)
                t2 = per.tile([128, KC * X2], BF16, name=nm2, tag=nm2)
                v1 = [t1[:, k * X1:(k + 1) * X1] for k in range(KC)]
                v2 = [t2[:, k * X2:(k + 1) * X2] for k in range(KC)]
                for k in range(KC):
                    nc.sync.dma_start(v1[k], h1[bass.ts(k, 128), :])
                    nc.sync.dma_start(v2[k], h2[bass.ts(k, 128), :])
                return v1, v2

            wqt, xqb = load_pair_interleaved(wq, D, "wqt", xtq_bf16, TH, "xqbt")
            wkt = load_chunked(wk, D, BF16, "wkt", eng=nc.gpsimd)
            xtb = load_chunked(xt_bf16, T, BF16, "xtbt", eng=nc.gpsimd)
            early = tc.alloc_tile_pool(name="early", bufs=1)

            def load_chunked_early(handle, X, dt, nm, nchunk=KC):
                tl = early.tile([128, nchunk * X], dt, name=nm, tag=nm)
                nc.gpsimd.dma_start(
                    tl[:].rearrange("p (c x) -> p c x", c=nchunk),
                    handle.ap().rearrange("(c p) x -> p c x", p=128),
                )
                return [tl[:, k * X:(k + 1) * X] for k in range(nchunk)]

            wst = load_chunked_early(ws, H, F32, "wst")
            xqf = load_chunked_early(xtq_f32, TH, F32, "xqft")
            wvt = load_chunked(wv, D, BF16, "wvt", eng=nc.gpsimd)
            nd2t = load_chunked(nd2, BW, F32, "nd2t", nchunk=NTC, eng=nc.gpsimd)
            wot = load_chunked(wo, D, BF16, "wot", eng=nc.gpsimd)
            bqt = per.tile([128, KC], F32, name="bq", tag="bq")
            bkt = per.tile([128, KC], F32, name="bk", tag="bk")
            bvt = per.tile([1, D], BF16, name="bv", tag="bv")
            bot = per.tile([1, D], BF16, name="bo", tag="bo")
            bsc = per.tile([H, 1], F32, name="bsc", tag="bsc")
            bsr = per.tile([1, H], F32, name="bsr", tag="bsr")
            nc.sync.dma_start(bqt[:], bq[:])
            nc.sync.dma_start(bkt[:], bk[:])
            nc.sync.dma_start(bvt[:], bv[:])
            nc.sync.dma_start(bot[:], bo[:])
            nc.sync.dma_start(bsc[:], bs_col[:])
            nc.sync.dma_start(bsr[:], bs_row[:])

            eye = per.tile([128, 128], BF16, name="eye", tag="eye")
            make_identity(nc, eye[:])
            ones_b = per.tile([1, 128], BF16, name="ones_b", tag="ones_b")
            nc.vector.memset(ones_b[:], 1.0)
            ones_f = per.tile([1, 128], F32, name="ones_f", tag="ones_f")
            nc.vector.memset(ones_f[:], 1.0)
            # pin the natural_log_exp_and_others ACT table set (has both exp
            # and ln) before any Exp, so walrus never switches sets mid-kernel
            tpin = per.tile([1, 1], F32, name="tpin", tag="tpin")
            nc.scalar.activation(tpin[:], ones_f[0:1, 0:1], AF.Ln)

            # ---- projections (emitted interleaved with head pairs) ----
            # q^T [dout, t_half] (f32r), k^T [dout, s_full] (f32r)
            qT = [per.tile([128, TH], F32R, name=f"qT{m}", tag=f"qT{m}") for m in range(KC)]
            kT = [per.tile([128, T], F32R, name=f"kT{m}", tag=f"kT{m}") for m in range(KC)]
            vt = [per.tile([128, D], BF16, name=f"v{s}", tag=f"v{s}") for s in range(NSC)]

            def proj_qk(m):
                ps = pmid.tile([128, 512], F32, name="mid", tag="mid")
                for k in range(KC):
                    nc.tensor.matmul(
                        ps[:], wqt[k][:, bass.ts(m, 128)], xqb[k][:],
                        start=(k == 0), stop=(k == KC - 1),
                    )
                nc.vector.tensor_scalar_add(qT[m][:], ps[:], bqt[:, m:m + 1])
                for sh in range(2):
                    ps2 = pmid.tile([128, 512], F32, name="mid", tag="mid")
                    for k in range(KC):
                        nc.tensor.matmul(
                            ps2[:], wkt[k][:, bass.ts(m, 128)],
                            xtb[k][:, bass.ts(sh, 512)],
                            start=(k == 0), stop=(k == KC - 1),
                        )
                    nc.vector.tensor_scalar_add(
                        kT[m][:, bass.ts(sh, 512)], ps2[:], bkt[:, m:m + 1]
                    )

            def proj_v(s):
                ps = pmid.tile([128, 512], F32, name="mid", tag="mid")
                for k in range(KC):
                    nc.tensor.matmul(
                        ps[:], xtb[k][:, bass.ts(s, 128)], wvt[k][:],
                        start=(k == 0), stop=False,
                    )
                nc.tensor.matmul(ps[:], ones_b[:], bvt[:], start=False, stop=True)
                nc.vector.tensor_copy(vt[s][:], ps[:])

            # ---- sigma (both orientations) ----------------------------
            # row orientation [H, TH] for the sigma output
            ps = pmid.tile([H, 512], F32, name="mid", tag="mid")
            for k in range(KC):
                nc.tensor.matmul(
                    ps[:], wst[k][:], xqf[k][:],
                    start=(k == 0), stop=(k == KC - 1),
                )
            # softplus(z) = ln(exp(z)+1); all Exps grouped before all Lns so
            # the ACT table set (natural_log_exp_and_others) loads once
            ez_row = early.tile([H, TH], F32, name="ez_row", tag="ez_row")
            nc.scalar.activation(ez_row[:], ps[:], AF.Exp, bias=bsc[:, 0:1])
            # natural orientation [t, H] -> inv2s2 = 1/(2*((sp+1e-6)^2+1e-6))
            inv2s2 = [per.tile([128, H], F32, name=f"i2s{t}", tag=f"i2s{t}") for t in range(NTC)]
            ezn = [early.tile([128, H], F32, name=f"ezn{t}", tag=f"ezn{t}") for t in range(NTC)]
            for t in range(NTC):
                psn = pmid.tile([128, 512], F32, name="mid", tag="mid")
                for k in range(KC):
                    nc.tensor.matmul(
                        psn[:, 0:H], xqf[k][:, bass.ts(t, 128)], wst[k][:],
                        start=(k == 0), stop=False,
                    )
                nc.tensor.matmul(psn[:, 0:H], ones_f[:], bsr[:], start=False, stop=True)
                nc.scalar.activation(ezn[t][:], psn[:, 0:H], AF.Exp)
            nc.scalar.activation(ez_row[:], ez_row[:], AF.Ln, bias=1.0)
            nc.vector.tensor_scalar_add(ez_row[:], ez_row[:], 1e-6)
            nc.sync.dma_start(o_sigma[:], ez_row[:])
            for t in range(NTC):
                sp = ezn[t]
                nc.scalar.activation(sp[:], ezn[t][:], AF.Ln, bias=1.0)
                nc.vector.tensor_scalar_add(sp[:], sp[:], 1e-6)
                sq = early.tile([128, H], F32, name=f"sq{t}", tag=f"sq{t}")
                nc.vector.tensor_tensor(sq[:], sp[:], sp[:], mybir.AluOpType.mult)
                nc.vector.tensor_scalar(
                    sq[:], sq[:], 1e-6, 2.0,
                    mybir.AluOpType.add, mybir.AluOpType.mult,
                )
                nc.vector.reciprocal(inv2s2[t][:], sq[:])

            early.release()

            # ---- per-head-pair attention + prior ----------------------
            # heads (2m, 2m+1) live in qT[m]/kT[m] at partition rows 0:64/64:128
            at = [per.tile([128, TH], BF16, name=f"at{m}", tag=f"at{m}") for m in range(KC)]

            def phase_logits(hp):
                """logits -> exp(+rowsum) -> series/prior normalize + DMA.
                Returns the pair's S' and diag tiles for phase_sv."""
                sp_t = [[None] * NTC for _ in range(2)]
                dg = [[None] * NTC for _ in range(2)]
                sst_p = [None, None]
                pst_p = [None, None]
                for t in range(NTC):
                    for side in range(2):
                        h, hr = 2 * hp + side, side * 64
                        lp = pbig.tile([128, T], F32, name="big", tag="big")
                        for sh in range(2):
                            nc.tensor.matmul(
                                lp[:, bass.ts(sh, 512)],
                                qT[hp][hr:hr + 64, bass.ts(t, 128)],
                                kT[hp][hr:hr + 64, bass.ts(sh, 512)],
                                start=True, stop=True,
                            )
                        spt = work.tile([128, T], BF16, name=f"sprime{side}_{t}", tag=f"sprime{side}_{t}")
                        sp_t[side][t] = spt
                        rs = work.tile([128, 1], F32, name=f"rs{side}_{t}", tag=f"rs{side}_{t}")
                        nc.scalar.activation(spt[:], lp[:], AF.Exp, accum_out=rs[:])
                        rc = work.tile([128, 1], F32, name=f"rc{side}_{t}", tag=f"rc{side}_{t}")
                        nc.vector.reciprocal(rc[:], rs[:])
                        dgt = work.tile([128, 128], BF16, name=f"diag{side}_{t}", tag=f"diag{side}_{t}")
                        dg[side][t] = dgt
                        nc.vector.tensor_scalar_mul(dgt[:], eye[:], rc[:, 0:1])
                        if t % 2 == 0:
                            sst_p[side] = stage.tile([128, 2 * T], F32, name="series_st", tag="series_st", bufs=3)
                        sst = sst_p[side]
                        nc.vector.tensor_scalar_mul(
                            sst[:, (t % 2) * T:(t % 2 + 1) * T], spt[:], rc[:, 0:1])
                        if t % 2 == 1:
                            nc.sync.dma_start(
                                o_series[h, (t - 1) * 128:(t + 1) * 128, :]
                                .rearrange("(c p) x -> p c x", p=128),
                                sst[:].rearrange("p (c x) -> p c x", c=2),
                            )
                        # prior for this (h, t-chunk), band only
                        pp = work.tile([128, BW], BF16, name=f"pp{side}_{t}", tag=f"pp{side}_{t}")
                        prs = work.tile([128, 1], F32, name=f"prs{side}_{t}", tag=f"prs{side}_{t}")
                        nc.scalar.activation(
                            pp[:], nd2t[t][:], AF.Exp,
                            scale=inv2s2[t][:, h:h + 1], accum_out=prs[:],
                        )
                        nc.vector.tensor_scalar_add(prs[:], prs[:], 1e-9)
                        prc = work.tile([128, 1], F32, name=f"prc{side}_{t}", tag=f"prc{side}_{t}")
                        nc.vector.reciprocal(prc[:], prs[:])
                        if t % 2 == 0:
                            pst_p[side] = stage.tile([128, 2 * BW], F32, name="prior_st", tag="prior_st", bufs=3)
                        pst = pst_p[side]
                        nc.vector.tensor_scalar_mul(
                            pst[:, (t % 2) * BW:(t % 2 + 1) * BW], pp[:], prc[:, 0:1])
                        if t % 2 == 1:
                            nc.sync.dma_start(
                                o_prior[h, (t - 1) * 128:(t + 1) * 128, :]
                                .rearrange("(c p) x -> p c x", p=128),
                                pst[:].rearrange("p (c x) -> p c x", c=2),
                            )
                return sp_t, dg

            def phase_sv(hp, sp_t, dg):
                """normalized transpose St = S'^T @ diag(1/rowsum), then the
                pair's S@v packed into one psum via col tile_position"""
                st = [[None] * NSC for _ in range(2)]
                ohp = pmid.tile([128, 512], F32, name="ohp", tag="oh", bufs=1)

                def sv(s):
                    for side in range(2):
                        h = 2 * hp + side
                        nc.tensor.matmul(
                            ohp[side * 64:side * 64 + 64, :],
                            vt[s][:, h * 64:h * 64 + 64], st[side][s][:],
                            start=(s == 0), stop=(s == NSC - 1),
                            skip_group_check=True,
                            tile_position=(0, side * 64),
                        )

                for s in range(NSC):
                    for side in range(2):
                        tp = pmid.tile([128, 512], F32, name="tp", tag="mid")
                        for t in range(NTC):
                            nc.tensor.matmul(
                                tp[:, bass.ts(t, 128)],
                                sp_t[side][t][:, bass.ts(s, 128)], dg[side][t][:],
                                start=True, stop=True, skip_group_check=True,
                            )
                        stt = work.tile([128, TH], BF16, name=f"st{side}_{s}", tag=f"st{side}_{s}")
                        st[side][s] = stt
                        if s % 2 == 0:
                            nc.vector.tensor_copy(stt[:], tp[:])
                        else:
                            nc.scalar.copy(stt[:], tp[:])
                    # S@v runs one s-chunk behind the transposes so the PE
                    # never stalls on the St psum->sbuf copy drain
                    if s > 0:
                        sv(s - 1)
                sv(NSC - 1)
                nc.vector.tensor_copy(at[hp][:], ohp[:])

            # software pipeline: only chunk 0 of q/k is projected up
            # front; remaining projections and v interleave with the head
            # pairs, and pair hp's logits/exp are emitted before pair
            # hp-1's transpose+S@v so every engine always has queued work
            proj_qk(0)
            prev = None
            for hp in range(H // 2):
                cur = (hp, *phase_logits(hp))
                if hp + 1 < KC:
                    proj_qk(hp + 1)
                if hp == 0:
                    for sc in range(NSC):
                        proj_v(sc)
                if prev is not None:
                    phase_sv(*prev)
                prev = cur
            phase_sv(*prev)

            # ---- output projection ------------------------------------
            ofull = stage.tile([128, NTC * D], F32, name="out_st", tag="out_st", bufs=1)
            for t in range(NTC):
                ps = pmid.tile([128, 512], F32, name="mid", tag="mid")
                for k in range(KC):
                    nc.tensor.matmul(
                        ps[:], at[k][:, bass.ts(t, 128)], wot[k][:],
                        start=(k == 0), stop=False,
                    )
                nc.tensor.matmul(ps[:], ones_b[:], bot[:], start=False, stop=True)
                nc.vector.tensor_copy(ofull[:, bass.ts(t, D)], ps[:])
            nc.sync.dma_start(
                o_out.ap().rearrange("(c p) x -> p c x", p=128),
                ofull[:].rearrange("p (c x) -> p c x", c=NTC),
            )

    nc.compile()
    return nc


def kernel(x, Wq_w, Wq_b, Wk_w, Wk_b, Wv_w, Wv_b, Ws_w, Ws_b, Wo_w, Wo_b):
    global LAST_RESULTS
    x = np.asarray(x, np.float32)
    Wq_w = np.asarray(Wq_w, np.float32); Wq_b = np.asarray(Wq_b, np.float32)
    Wk_w = np.asarray(Wk_w, np.float32); Wk_b = np.asarray(Wk_b, np.float32)
    Wv_w = np.asarray(Wv_w, np.float32); Wv_b = np.asarray(Wv_b, np.float32)
    Ws_w = np.asarray(Ws_w, np.float32); Ws_b = np.asarray(Ws_b, np.float32)
    Wo_w = np.asarray(Wo_w, np.float32); Wo_b = np.asarray(Wo_b, np.float32)

    if "nc" not in _NC_CACHE:
        _NC_CACHE["nc"] = _build()
    nc = _NC_CACHE["nc"]

    scale = 1.0 / np.sqrt(DH, dtype=np.float32)  # folded via q-side: 1/8
    wq_s = (Wq_w * scale).astype(ml_dtypes.bfloat16)
    bq_s = (Wq_b * scale).reshape(KC, 128).T.copy()
    wk_c = Wk_w.astype(ml_dtypes.bfloat16)
    bk_c = Wk_b.reshape(KC, 128).T.copy()
    wv_c = Wv_w.astype(ml_dtypes.bfloat16)
    bv_c = Wv_b.reshape(1, D).astype(ml_dtypes.bfloat16)
    wo_c = Wo_w.astype(ml_dtypes.bfloat16)
    bo_c = Wo_b.reshape(1, D).astype(ml_dtypes.bfloat16)
    bs_col = Ws_b.reshape(H, 1).astype(np.float32)
    bs_row = Ws_b.reshape(1, H).astype(np.float32)

    idx = np.arange(T, dtype=np.float32)

    def band_start(t0, tc):
        return min(max(t0 + tc * 128 - 128, 0), T - BW)

    def make_nd2_band(t0):
        nd2b = np.empty((TH, BW), np.float32)
        for tc in range(NTC):
            c0 = band_start(t0, tc)
            rows = t0 + tc * 128 + np.arange(128, dtype=np.float32)
            cols = c0 + np.arange(BW, dtype=np.float32)
            nd2b[tc * 128:(tc + 1) * 128, :] = -((rows[:, None] - cols[None, :]) ** 2)
        return nd2b

    in_maps = []
    for c in range(8):
        b, h2 = c // 2, c % 2
        t0 = h2 * TH
        xt = np.ascontiguousarray(x[b].T)
        in_maps.append({
            "xt_bf16": xt.astype(ml_dtypes.bfloat16),
            "xtq_f32": np.ascontiguousarray(xt[:, t0:t0 + TH]),
            "xtq_bf16": np.ascontiguousarray(xt[:, t0:t0 + TH]).astype(ml_dtypes.bfloat16),
            "wq": wq_s, "wk": wk_c, "wv": wv_c, "wo": wo_c,
            "ws": Ws_w,
            "bq": bq_s, "bk": bk_c, "bv": bv_c, "bo": bo_c,
            "bs_col": bs_col, "bs_row": bs_row,
            "nd2": make_nd2_band(t0),
        })

    trace = os.environ.get("BASS_KERNEL_TRACE") == "1"
    res = run_bass_kernel_spmd(nc, in_maps, core_ids=list(range(8)), trace=trace)
    LAST_RESULTS = res

    out = np.empty((B, T, D), np.float32)
    series = np.empty((B, H, T, T), np.float32)
    prior = np.zeros((B, H, T, T), np.float32)
    sigma = np.empty((B, H, T), np.float32)
    for c in range(8):
        b, h2 = c // 2, c % 2
        t0 = h2 * TH
        r = res.results[c]
        out[b, t0:t0 + TH, :] = r["out"]
        series[b, :, t0:t0 + TH, :] = r["series"]
        pb = r["prior"]
        for tc in range(NTC):
            c0 = band_start(t0, tc)
            g0 = t0 + tc * 128
            prior[b, :, g0:g0 + 128, c0:c0 + BW] = pb[:, tc * 128:(tc + 1) * 128, :]
        sigma[b, :, t0:t0 + TH] = r["sigma"]
    return out, series, prior, sigma
